# revision 1
# baseline (speedup 1.0000x reference)
"""Trainium2 Bass kernel for nn_DTFDynamicLayer (moe_routing).

Self-contained: takes FULL inputs, returns FULL output. Two SPMD NEFFs on 8
NeuronCores:
  NEFF1 (scoring): token-parallel router scores (fp32 — top-k boundaries are
    ~6e-4 apart, bf16 would flip selections).
  Host: top-k per batch row (argpartition), gather selected tokens.
  NEFF2 (decoder): tensor-parallel Qwen2 block on the T=B*k=2048 selected
    tokens. Per core: 2 Q heads + its KV head, 1/8 of d_ff; o-proj and
    down-proj emit full-D partials ReduceScattered over cores; x1 slices
    AllGathered (bf16) for the MLP. RMS ln weights folded into the weight
    slices; rms r1 and rope tables folded host-side; r2 computed on device.
  Host: scatter-add gated delta into a copy of hidden_states.
"""
import sys
sys.path.insert(0, "/opt/trn_rl_repo")
import math
import numpy as np
import ml_dtypes

import jax
from jax.sharding import Mesh, PartitionSpec
from jax.experimental.shard_map import shard_map

import concourse.bacc as bacc
import concourse.mybir as mybir
import concourse.tile as tile
from concourse.bass_utils import run_bass_kernel_spmd
from concourse.bass2jax import (_bass_exec_p, partition_id_tensor,
                                install_neuronx_cc_hook)


def _make_runner(nc, n_cores=8):
    """Persistent jitted shard_map executor for an SPMD Bass program."""
    install_neuronx_cc_hook()
    pname = nc.partition_id_tensor.name if nc.partition_id_tensor else None
    in_names, out_names, out_avals, zero_outs = [], [], [], []
    for alloc in nc.m.functions[0].allocations:
        if not isinstance(alloc, mybir.MemoryLocationSet):
            continue
        name = alloc.memorylocations[0].name
        if alloc.kind == "ExternalInput":
            if name != pname:
                in_names.append(name)
        elif alloc.kind == "ExternalOutput":
            shape = tuple(alloc.tensor_shape)
            dtype = mybir.dt.np(alloc.dtype)
            out_names.append(name)
            out_avals.append(jax.core.ShapedArray(shape, dtype))
            zero_outs.append(np.zeros(shape, dtype))
    all_in = list(in_names) + list(out_names)
    if pname is not None:
        all_in.append(pname)

    def _body(*args):
        operands = list(args)
        if pname is not None:
            operands.append(partition_id_tensor())
        return tuple(_bass_exec_p.bind(
            *operands, out_avals=tuple(out_avals), in_names=tuple(all_in),
            out_names=tuple(out_names), lowering_input_output_aliases=(),
            sim_require_finite=True, sim_require_nnan=True, nc=nc))

    devices = jax.devices()[:n_cores]
    mesh = Mesh(np.asarray(devices), ("core",))
    nin = len(in_names) + len(out_avals)
    sharded = jax.jit(
        shard_map(_body, mesh=mesh,
                  in_specs=(PartitionSpec("core"),) * nin,
                  out_specs=(PartitionSpec("core"),) * len(out_avals),
                  check_rep=False),
        keep_unused=True)
    concat_zeros = [np.zeros((n_cores * z.shape[0], *z.shape[1:]), z.dtype)
                    for z in zero_outs]

    def run(in_maps):
        concat_in = [np.concatenate([np.asarray(in_maps[c][nm])
                                     for c in range(n_cores)], axis=0)
                     for nm in in_names]
        outs = sharded(*concat_in, *concat_zeros)
        return [{nm: np.asarray(outs[i]).reshape(n_cores, *out_avals[i].shape)[c]
                 for i, nm in enumerate(out_names)}
                for c in range(n_cores)]

    return run

N_CORES = 8
B, S = 4, 4096
T = 2048
D = 2048
HD = 128
DFF = 8192
NK = D // 128
TC = T // 512
NTOK = T // 128
DSL = D // N_CORES
DFFL = DFF // N_CORES
TOKC = B * S // N_CORES      # scoring tokens per core
NCH = TOKC // 128
EPS = 1e-6
SM_SCALE = 1.0 / math.sqrt(HD)
ROPE_THETA = 10000.0

f32 = mybir.dt.float32
bf16 = mybir.dt.bfloat16
AF = mybir.ActivationFunctionType
OP = mybir.AluOpType
BF = ml_dtypes.bfloat16

_cache = {}
BENCH_MODE = False


def _ext_in(nc, name, shape, dtype, reg):
    """ExternalInput normally; internal DRAM tensor in BENCH_MODE."""
    if BENCH_MODE:
        t = nc.dram_tensor(name, shape, dtype)
        reg.append(t)
        return t
    return nc.dram_tensor(name, shape, dtype, kind="ExternalInput")


def _bench_init(nc, tc, pool, reg):
    """Fill fake-input internal tensors with a small constant (bench mode)."""
    if not reg:
        return
    zt = pool.tile([128, 2048], bf16, tag="bigbf", name="benchinit")
    nc.vector.memset(zt[:], 0.01)
    for t in reg:
        rows, cols = t.shape
        for r in range(0, rows, 128):
            rr = min(128, rows - r)
            for cstart in range(0, cols, 2048):
                cc = min(2048, cols - cstart)
                nc.gpsimd.dma_start(out=t[r:r + rr, cstart:cstart + cc],
                                    in_=zt[0:rr, 0:cc])


# ======================= NEFF1: scoring =======================
def build_scoring():
    nc = bacc.Bacc("TRN2", target_bir_lowering=False, debug=False,
                   num_devices=N_CORES)
    breg = []
    orig = _ext_in(nc, "orig", [TOKC, D], f32, breg)
    dsur = _ext_in(nc, "dsur", [TOKC, D], f32, breg)
    wb = _ext_in(nc, "wb", [128, D], f32, breg)
    scores = nc.dram_tensor("scores", [TOKC], f32, kind="ExternalOutput")

    with tile.TileContext(nc) as tc:
        with tc.tile_pool(name="io", bufs=3) as io, \
             tc.tile_pool(name="scratch", bufs=2) as scratch, \
             tc.tile_pool(name="bigbf0", bufs=1) as bigbf0, \
             tc.tile_pool(name="acc", bufs=1) as accp:
            _bench_init(nc, tc, bigbf0, breg)
            wb_t = accp.tile([128, D], f32)
            nc.sync.dma_start(out=wb_t[:], in_=wb[:, :])
            ss = accp.tile([128, NCH], f32, tag="ss")
            dot = accp.tile([128, NCH], f32, tag="dot")

            for i in range(NCH):
                o_t = io.tile([128, D], f32, tag="o", name=f"o{i}")
                d_t = io.tile([128, D], f32, tag="p", name=f"p{i}")
                sl = slice(i * 128, (i + 1) * 128)
                nc.sync.dma_start(out=o_t[:], in_=orig[sl, :])
                nc.scalar.dma_start(out=d_t[:], in_=dsur[sl, :])

                sq_t = scratch.tile([128, D], bf16, tag="sq", name=f"sq{i}")
                nc.scalar.activation(sq_t[:], d_t[:], AF.Square,
                                     accum_out=ss[:, i:i + 1])
                pr_t = scratch.tile([128, D], bf16, tag="pr", name=f"pr{i}")
                nc.vector.scalar_tensor_tensor(pr_t[:], o_t[:], 1.0, wb_t[:],
                                               op0=OP.mult, op1=OP.mult,
                                               accum_out=dot[:, i:i + 1])

            sc = accp.tile([128, NCH], f32, tag="sc")
            nc.vector.scalar_tensor_tensor(sc[:], ss[:], 0.5 / D, dot[:],
                                           op0=OP.mult, op1=OP.add)
            nc.sync.dma_start(out=scores.rearrange("(n p) -> p n", p=128),
                              in_=sc[:])
    nc.compile()
    return nc


# ======================= NEFF2: decoder =======================
def build_decoder():
    nc = bacc.Bacc("TRN2", target_bir_lowering=False, debug=False,
                   num_devices=N_CORES)
    breg = []
    xT = _ext_in(nc, "xT", [D, T], bf16, breg)
    wq_in = _ext_in(nc, "wq_in", [128, NK * 256], bf16, breg)
    wk_in = _ext_in(nc, "wk_in", [128, NK * 128], bf16, breg)
    wv_in = _ext_in(nc, "wv_in", [128, NK * 128], bf16, breg)
    wo_in = _ext_in(nc, "wo_in", [128, 2 * D], bf16, breg)
    wg_in = _ext_in(nc, "wg_in", [128, 8 * D], bf16, breg)
    wu_in = _ext_in(nc, "wu_in", [128, 8 * D], bf16, breg)
    wd_in = _ext_in(nc, "wd_in", [128, 16 * DFFL], bf16, breg)
    cos_in = _ext_in(nc, "cos_in", [128, T], bf16, breg)
    sin_in = _ext_in(nc, "sin_in", [128, T], bf16, breg)
    r1c_in = _ext_in(nc, "r1c_in", [128, NTOK], f32, breg)
    gate_in = _ext_in(nc, "gate_in", [128, T], bf16, breg)
    delta = nc.dram_tensor("delta", [DSL, T], f32, kind="ExternalOutput")
    o_part = nc.dram_tensor("o_part", [D, T], bf16)
    o_allred = nc.dram_tensor("o_allred", [D, T], bf16, addr_space="Shared")
    p_d = nc.dram_tensor("p_d", [DFFL, T], bf16)
    mlp_part = nc.dram_tensor("mlp_part", [D, T], bf16)
    mlp_red = nc.dram_tensor("mlp_red", [DSL, T], bf16)
    RG = [list(range(N_CORES))]

    with tile.TileContext(nc) as tc:
        with tc.tile_pool(name="wres", bufs=1) as wres, \
             tc.tile_pool(name="xch", bufs=NK) as xch, \
             tc.tile_pool(name="att", bufs=1) as att, \
             tc.tile_pool(name="wstream", bufs=2) as wstream, \
             tc.tile_pool(name="big1", bufs=2) as big1, \
             tc.tile_pool(name="bigbf", bufs=3) as bigbf, \
             tc.tile_pool(name="sm", bufs=3) as sm, \
             tc.tile_pool(name="pn", bufs=1) as pnp, \
             tc.tile_pool(name="ev", bufs=3) as ev, \
             tc.tile_pool(name="ps", bufs=4, space="PSUM") as ps, \
             tc.tile_pool(name="psden", bufs=4, space="PSUM") as psden:

            _bench_init(nc, tc, bigbf, breg)
            wq_sb = wres.tile([128, NK * 256], bf16, tag="wq")
            nc.sync.dma_start(out=wq_sb[:], in_=wq_in[:, :])
            wk_sb = wres.tile([128, NK * 128], bf16, tag="wk")
            nc.sync.dma_start(out=wk_sb[:], in_=wk_in[:, :])
            wv_sb = wres.tile([128, NK * 128], bf16, tag="wv")
            nc.sync.dma_start(out=wv_sb[:], in_=wv_in[:, :])
            wo_sb = wres.tile([128, 2 * D], bf16, tag="wo")
            nc.sync.dma_start(out=wo_sb[:], in_=wo_in[:, :])
            cos_sb = wres.tile([128, T], bf16, tag="cos")
            nc.sync.dma_start(out=cos_sb[:], in_=cos_in[:, :])
            sin_sb = wres.tile([128, T], bf16, tag="sin")
            nc.sync.dma_start(out=sin_sb[:], in_=sin_in[:, :])
            r1c_sb = wres.tile([128, NTOK], f32, tag="r1c")
            nc.sync.dma_start(out=r1c_sb[:], in_=r1c_in[:, :])
            ones_sb = wres.tile([128, 1], bf16, tag="ones")
            nc.vector.memset(ones_sb[:], 1.0)
            r2row = wres.tile([1, T], bf16, tag="r2row")

            x_sb = []
            for k in range(NK):
                xk = xch.tile([128, T], bf16, tag="x", name=f"x{k}")
                eng = nc.sync if k % 2 == 0 else nc.scalar
                eng.dma_start(out=xk[:], in_=xT[k * 128:(k + 1) * 128, :])
                x_sb.append(xk)

            # ---- Q/K projections ----
            qraw = [att.tile([128, T], bf16, tag=f"qo{h}", name=f"qraw{h}")
                    for h in range(2)]
            kraw = att.tile([128, T], bf16, tag="kr2")
            for h in range(2):
                for n in range(TC):
                    acc = ps.tile([128, 512], f32, tag="acc", name=f"qa{h}{n}")
                    for k in range(NK):
                        nc.tensor.matmul(
                            acc[:],
                            wq_sb[:, k * 256 + h * 128: k * 256 + (h + 1) * 128],
                            x_sb[k][:, n * 512:(n + 1) * 512],
                            start=(k == 0), stop=(k == NK - 1))
                    nc.scalar.copy(qraw[h][:, n * 512:(n + 1) * 512], acc[:])
            for n in range(TC):
                acc = ps.tile([128, 512], f32, tag="acc", name=f"ka{n}")
                for k in range(NK):
                    nc.tensor.matmul(
                        acc[:], wk_sb[:, k * 128:(k + 1) * 128],
                        x_sb[k][:, n * 512:(n + 1) * 512],
                        start=(k == 0), stop=(k == NK - 1))
                nc.scalar.copy(kraw[:, n * 512:(n + 1) * 512], acc[:])

            # ---- V projection (r1 fused) ----
            v_sb = att.tile([128, NTOK * 128], bf16, tag="vg")
            for j in range(NTOK):
                acc = ps.tile([128, 512], f32, tag="acc", name=f"va{j}")
                for k in range(NK):
                    nc.tensor.matmul(
                        acc[:, 0:128], x_sb[k][:, j * 128:(j + 1) * 128],
                        wv_sb[:, k * 128:(k + 1) * 128],
                        start=(k == 0), stop=(k == NK - 1))
                nc.vector.tensor_scalar(
                    v_sb[:, j * 128:(j + 1) * 128], acc[:, 0:128],
                    r1c_sb[:, j:j + 1], None, op0=OP.mult)

            # ---- rope ----
            qt = [att.tile([128, T], bf16, tag=f"qt{h}", name=f"qt{h}")
                  for h in range(2)]
            kt = att.tile([128, T], bf16, tag="kt")
            for raw, out_t in ((qraw[0], qt[0]), (qraw[1], qt[1]), (kraw, kt)):
                swp = bigbf.tile([128, T], bf16, tag="bigbf", name="swp")
                nc.sync.dma_start(out=swp[0:64, :], in_=raw[64:128, :])
                nc.sync.dma_start(out=swp[64:128, :], in_=raw[0:64, :])
                t1 = bigbf.tile([128, T], bf16, tag="bigbf", name="t1")
                nc.vector.tensor_tensor(t1[:], raw[:], cos_sb[:], op=OP.mult)
                t2 = bigbf.tile([128, T], bf16, tag="bigbf", name="t2")
                nc.vector.tensor_tensor(t2[:], swp[:], sin_sb[:], op=OP.mult)
                nc.vector.tensor_tensor(out_t[:], t1[:], t2[:], op=OP.add)

            # ---- attention ----
            ot = [att.tile([128, T], bf16, tag=f"qo{h}", name=f"ot{h}")
                  for h in range(2)]
            for h in range(2):
                for c in range(TC):
                    o_acc = ps.tile([128, 512], f32, tag="acc", name=f"oa{h}{c}")
                    den = psden.tile([1, 512], f32, tag="den", name=f"dn{h}{c}")
                    nj = 4 * c + 4
                    for j in range(nj):
                        s_ps = ps.tile([128, 512], f32, tag="acc",
                                       name=f"s{h}{c}{j}")
                        nc.tensor.matmul(
                            s_ps[:], kt[:, j * 128:(j + 1) * 128],
                            qt[h][:, c * 512:(c + 1) * 512],
                            start=True, stop=True)
                        e = sm.tile([128, 512], bf16, tag="exp", name="e")
                        nc.scalar.activation(e[:], s_ps[:], AF.Exp,
                                             scale=SM_SCALE)
                        if j >= 4 * c:
                            v_ = j - 4 * c
                            e2 = sm.tile([128, 512], bf16, tag="exp", name="e2")
                            nc.gpsimd.affine_select(
                                e2[:], e[:], pattern=[[1, 512]],
                                compare_op=OP.is_ge, fill=0.0,
                                base=-v_ * 128, channel_multiplier=-1)
                            e = e2
                        nc.tensor.matmul(o_acc[:], v_sb[:, j * 128:(j + 1) * 128],
                                         e[:], start=(j == 0), stop=(j == nj - 1))
                        nc.tensor.matmul(den[:], ones_sb[:], e[:],
                                         start=(j == 0), stop=(j == nj - 1))
                    den_sb = ev.tile([1, 512], f32, tag="densb", bufs=2,
                                     name="den_sb")
                    nc.vector.reciprocal(den_sb[:], den[:])
                    den_b = ev.tile([128, 512], f32, tag="denb", bufs=2,
                                    name="den_b")
                    nc.gpsimd.partition_broadcast(den_b[:], den_sb[:])
                    nc.vector.tensor_tensor(ot[h][:, c * 512:(c + 1) * 512],
                                            o_acc[:], den_b[:], op=OP.mult)

            # ---- o-proj partial -> DRAM -> ReduceScatter ----
            for m in range(NK):
                for n in range(TC):
                    acc = ps.tile([128, 512], f32, tag="acc", name=f"op{m}{n}")
                    for kk in range(2):
                        nc.tensor.matmul(
                            acc[:],
                            wo_sb[:, kk * D + m * 128: kk * D + (m + 1) * 128],
                            ot[kk][:, n * 512:(n + 1) * 512],
                            start=(kk == 0), stop=(kk == 1))
                    ob = ev.tile([128, 512], bf16, tag="evf32", name="ob", bufs=2)
                    nc.scalar.copy(ob[:], acc[:])
                    eng = nc.sync if (m + n) % 2 == 0 else nc.scalar
                    eng.dma_start(
                        out=o_part[m * 128:(m + 1) * 128, n * 512:(n + 1) * 512],
                        in_=ob[:])
            nc.gpsimd.collective_compute(
                "AllReduce", OP.add, replica_groups=RG,
                ins=[o_part[:, :]], outs=[o_allred[:, :]])

            # ---- x1 = x + o (in-place into x_sb) + r2 ----
            for k in range(NK):
                och = bigbf.tile([128, T], bf16, tag="bigbf", name=f"och{k}")
                eng = nc.sync if k % 2 == 0 else nc.scalar
                eng.dma_start(out=och[:], in_=o_allred[k * 128:(k + 1) * 128, :])
                nc.vector.tensor_tensor(x_sb[k][:], x_sb[k][:], och[:],
                                        op=OP.add)
            x1_sb = x_sb
            r2ps = [psden.tile([1, 512], f32, tag="den", name=f"r2ps{c}")
                    for c in range(TC)]
            for k in range(NK):
                sq = bigbf.tile([128, T], bf16, tag="bigbf", name=f"sq{k}")
                nc.scalar.activation(sq[:], x1_sb[k][:], AF.Square)
                for c in range(TC):
                    nc.tensor.matmul(r2ps[c][:], ones_sb[:],
                                     sq[:, c * 512:(c + 1) * 512],
                                     start=(k == 0), stop=(k == NK - 1))
            for c in range(TC):
                csl = slice(c * 512, (c + 1) * 512)
                mn = ev.tile([1, 512], f32, tag="densb", name=f"r2mn{c}", bufs=2)
                nc.vector.tensor_scalar(mn[:], r2ps[c][:], 1.0 / D, EPS,
                                        op0=OP.mult, op1=OP.add)
                rc = ev.tile([1, 512], f32, tag="densb", name=f"r2rc{c}", bufs=2)
                nc.vector.reciprocal(rc[:], mn[:])
                nc.scalar.activation(r2row[:, csl], rc[:], AF.Sqrt)
            r2b = att.tile([128, T], bf16, tag="kr2")
            nc.gpsimd.partition_broadcast(r2b[:], r2row[:])

            # ---- gate/up + silu -> P ----
            for m in range(8):
                wg_m = wstream.tile([128, D], bf16, tag="wgm", name=f"wg{m}")
                nc.scalar.dma_start(out=wg_m[:], in_=wg_in[:, m * D:(m + 1) * D])
                wu_m = wstream.tile([128, D], bf16, tag="wum", name=f"wu{m}")
                nc.scalar.dma_start(out=wu_m[:], in_=wu_in[:, m * D:(m + 1) * D])
                for n in range(TC):
                    gps = ps.tile([128, 512], f32, tag="acc", name=f"g{m}{n}")
                    for k in range(NK):
                        nc.tensor.matmul(gps[:], wg_m[:, k * 128:(k + 1) * 128],
                                         x1_sb[k][:, n * 512:(n + 1) * 512],
                                         start=(k == 0), stop=(k == NK - 1))
                    ups = ps.tile([128, 512], f32, tag="acc", name=f"u{m}{n}")
                    for k in range(NK):
                        nc.tensor.matmul(ups[:], wu_m[:, k * 128:(k + 1) * 128],
                                         x1_sb[k][:, n * 512:(n + 1) * 512],
                                         start=(k == 0), stop=(k == NK - 1))
                    gsc = sm.tile([128, 512], bf16, tag="gsc", bufs=2, name="gsc")
                    nc.vector.tensor_tensor(gsc[:], gps[:],
                                            r2b[:, n * 512:(n + 1) * 512],
                                            op=OP.mult)
                    gact = sm.tile([128, 512], bf16, tag="gact", bufs=2,
                                   name="gact")
                    nc.scalar.activation(gact[:], gsc[:], AF.Silu)
                    usc = sm.tile([128, 512], bf16, tag="usc", bufs=2, name="usc")
                    nc.vector.tensor_tensor(usc[:], ups[:],
                                            r2b[:, n * 512:(n + 1) * 512],
                                            op=OP.mult)
                    pmn = sm.tile([128, 512], bf16, tag="pmn", bufs=2, name="pmn")
                    nc.vector.tensor_tensor(pmn[:], gact[:], usc[:], op=OP.mult)
                    nc.scalar.dma_start(
                        out=p_d[m * 128:(m + 1) * 128, n * 512:(n + 1) * 512],
                        in_=pmn[:])

            # ---- down partial -> DRAM -> ReduceScatter (n-outer: P loaded
            # once per n and reused across all 16 m-tiles) ----
            for n in range(TC):
                pn = []
                for k in range(8):
                    pk = pnp.tile([128, 512], bf16, tag=f"pn{k}",
                                  name=f"pk{n}{k}", bufs=2)
                    eng = nc.sync if k % 2 == 0 else nc.scalar
                    eng.dma_start(
                        out=pk[:],
                        in_=p_d[k * 128:(k + 1) * 128, n * 512:(n + 1) * 512])
                    pn.append(pk)
                for m in range(NK):
                    wd_m = wstream.tile([128, DFFL], bf16, tag="wdm",
                                        name=f"wd{n}{m}")
                    nc.scalar.dma_start(out=wd_m[:],
                                        in_=wd_in[:, m * DFFL:(m + 1) * DFFL])
                    acc = ps.tile([128, 512], f32, tag="acc", name=f"d{n}{m}")
                    for k in range(8):
                        nc.tensor.matmul(acc[:], wd_m[:, k * 128:(k + 1) * 128],
                                         pn[k][:], start=(k == 0), stop=(k == 7))
                    oc2 = pnp.tile([128, 512], bf16, tag="och2", name="oc2",
                                   bufs=2)
                    nc.sync.dma_start(
                        out=oc2[:], in_=o_allred[m * 128:(m + 1) * 128,
                                                 n * 512:(n + 1) * 512])
                    db = ev.tile([128, 512], bf16, tag="evf32", name="db", bufs=2)
                    nc.vector.scalar_tensor_tensor(db[:], oc2[:], 0.125, acc[:],
                                                   op0=OP.mult, op1=OP.add)
                    nc.scalar.dma_start(
                        out=mlp_part[m * 128:(m + 1) * 128, n * 512:(n + 1) * 512],
                        in_=db[:])
            nc.gpsimd.collective_compute(
                "ReduceScatter", OP.add, replica_groups=RG,
                ins=[mlp_part[:, :]], outs=[mlp_red[:, :]])

            # ---- delta = (o_red + mlp_red) * gate ----
            gate_sb = att.tile([128, T], bf16, tag="vg")
            nc.sync.dma_start(out=gate_sb[:], in_=gate_in[:, :])
            for i in range(2):
                for n in range(TC):
                    csl = slice(n * 512, (n + 1) * 512)
                    msb = big1.tile([128, 512], bf16, tag="bigf1", name="msb")
                    nc.sync.dma_start(out=msb[:],
                                      in_=mlp_red[i * 128:(i + 1) * 128, csl])
                    dout = big1.tile([128, 512], f32, tag="bigf0", name="dout")
                    nc.vector.tensor_tensor(dout[:], msb[:], gate_sb[:, csl],
                                            op=OP.mult)
                    nc.sync.dma_start(out=delta[i * 128:(i + 1) * 128, csl],
                                      in_=dout[:])
    nc.compile()
    return nc


def _get(name, builder):
    if name not in _cache:
        _cache[name] = builder()
    return _cache[name]


def _tile_w(w, kchunks, mblocks=None):
    K, M = w.shape
    if mblocks is None:
        return np.ascontiguousarray(
            w.reshape(kchunks, 128, M).transpose(1, 0, 2)
            .reshape(128, kchunks * M))
    mb = M // mblocks
    return np.ascontiguousarray(
        w.reshape(kchunks, 128, mblocks, mb).transpose(1, 2, 0, 3)
        .reshape(128, mblocks * kchunks * mb))


def _run(nc, in_maps, trace=False):
    key = ("runner", id(nc))
    if key not in _cache:
        _cache[key] = _make_runner(nc)
    results = _cache[key](in_maps)

    class _R:
        pass

    r = _R()
    r.results = results
    r.exec_time_ns = None
    r.profile_json = None
    return r


def run_scoring(original, posterior, prior, w_router, trace=False):
    of = original.reshape(-1, D)
    df = (posterior.reshape(-1, D) - prior.reshape(-1, D))
    wb = np.ascontiguousarray(np.broadcast_to(w_router, (128, D)),
                              dtype=np.float32)
    in_maps = []
    for c in range(N_CORES):
        sl = slice(c * TOKC, (c + 1) * TOKC)
        in_maps.append(dict(orig=of[sl], dsur=df[sl], wb=wb))
    res = _run(_get("scoring", build_scoring), in_maps, trace)
    out = np.concatenate([res.results[c]["scores"] for c in range(N_CORES)])
    return out.reshape(B, S), res


def prep_decoder_in_maps(sel, pos, gate, ln1_w, wq, wk, wv, wo, ln2_w,
                         w_gate, w_up, w_down):
    r1 = 1.0 / np.sqrt((sel.astype(np.float32) ** 2).mean(-1) + EPS)
    xT_bf = sel.T.astype(BF)

    inv_freq = 1.0 / (ROPE_THETA ** (np.arange(0, HD, 2, dtype=np.float32) / HD))
    ang = pos[:, None].astype(np.float32) * inv_freq[None, :]
    cos_v = np.cos(ang).T * r1[None, :]
    sin_v = np.sin(ang).T * r1[None, :]
    cos_t = np.ascontiguousarray(np.concatenate([cos_v, cos_v], 0).astype(BF))
    sin_t = np.ascontiguousarray(np.concatenate([-sin_v, sin_v], 0).astype(BF))
    r1c = np.ascontiguousarray(r1.reshape(NTOK, 128).T.astype(np.float32))
    gate_b = np.ascontiguousarray(np.broadcast_to(gate.astype(BF), (128, T)))

    wq_f = (ln1_w[:, None] * wq).astype(np.float32)
    wk_f = (ln1_w[:, None] * wk).astype(np.float32)
    wv_f = (ln1_w[:, None] * wv).astype(np.float32)
    wg_f = (ln2_w[:, None] * w_gate).astype(np.float32)
    wu_f = (ln2_w[:, None] * w_up).astype(np.float32)

    in_maps = []
    for c in range(N_CORES):
        kvi = c // 2
        in_maps.append(dict(
            xT=np.ascontiguousarray(xT_bf),
            wq_in=_tile_w(wq_f[:, c * 256:(c + 1) * 256].astype(BF), NK),
            wk_in=_tile_w(wk_f[:, kvi * 128:(kvi + 1) * 128].astype(BF), NK),
            wv_in=_tile_w(wv_f[:, kvi * 128:(kvi + 1) * 128].astype(BF), NK),
            wo_in=_tile_w(np.asarray(wo, dtype=np.float32)
                          [c * 256:(c + 1) * 256].astype(BF), 2),
            wg_in=_tile_w(wg_f[:, c * DFFL:(c + 1) * DFFL].astype(BF), NK,
                          mblocks=8),
            wu_in=_tile_w(wu_f[:, c * DFFL:(c + 1) * DFFL].astype(BF), NK,
                          mblocks=8),
            wd_in=_tile_w(np.asarray(w_down, dtype=np.float32)
                          [c * DFFL:(c + 1) * DFFL].astype(BF), 8, mblocks=16),
            cos_in=cos_t, sin_in=sin_t, r1c_in=r1c, gate_in=gate_b,
        ))
    return in_maps


def run_decoder(sel, pos, gate, ln1_w, wq, wk, wv, wo, ln2_w, w_gate, w_up,
                w_down, trace=False):
    in_maps = prep_decoder_in_maps(sel, pos, gate, ln1_w, wq, wk, wv, wo,
                                   ln2_w, w_gate, w_up, w_down)
    res = _run(_get("decoder", build_decoder), in_maps, trace)
    delta_T = np.concatenate([res.results[c]["delta"] for c in range(N_CORES)],
                             0)
    return delta_T.T.astype(np.float32), res


def _kernel_numpy_fallback(hidden_states, original, posterior, prior,
                           position_ids, w_router, ln1_w, ln2_w, wq, wk, wv,
                           wo, w_gate, w_up, w_down, k):
    """Pure-numpy reference path (used only if shapes diverge from the spec)."""
    x = hidden_states.astype(np.float64)
    scores = (original.astype(np.float64) @ w_router.astype(np.float64)
              + 0.5 * ((posterior.astype(np.float64)
                        - prior.astype(np.float64)) ** 2).mean(-1))
    signal = 1.0 / (1.0 + np.exp(-scores))
    kk = int(k)
    idx = np.sort(np.argpartition(-scores, kk, axis=-1)[:, :kk], axis=-1)
    bidx = np.repeat(np.arange(x.shape[0]), kk)
    tidx = idx.reshape(-1)
    sel = x[bidx, tidx]
    gate = signal[bidx, tidx]
    pos = position_ids[bidx, tidx]
    Tl = sel.shape[0]
    H, KV = 16, 4

    def rms(v, w):
        return v / np.sqrt((v ** 2).mean(-1, keepdims=True) + EPS) * w

    h = rms(sel, ln1_w)
    q = (h @ wq).reshape(Tl, H, HD)
    k_ = (h @ wk).reshape(Tl, KV, HD)
    v_ = (h @ wv).reshape(Tl, KV, HD)
    inv_freq = 1.0 / (ROPE_THETA ** (np.arange(0, HD, 2) / HD))
    angv = pos[:, None] * inv_freq[None, :]
    cos = np.concatenate([np.cos(angv)] * 2, -1)[:, None, :]
    sin = np.concatenate([np.sin(angv)] * 2, -1)[:, None, :]

    def rope(t):
        t1, t2 = np.split(t, 2, -1)
        return t * cos + np.concatenate([-t2, t1], -1) * sin

    q, k_ = rope(q), rope(k_)
    k_ = np.repeat(k_, H // KV, 1)
    v_ = np.repeat(v_, H // KV, 1)
    att = np.einsum("thd,shd->hts", q, k_) / np.sqrt(HD)
    att = np.where(np.tril(np.ones((Tl, Tl), bool))[None], att, -1e9)
    att = np.exp(att - att.max(-1, keepdims=True))
    att /= att.sum(-1, keepdims=True)
    o = np.einsum("hts,shd->thd", att, v_).reshape(Tl, H * HD) @ wo
    x1 = sel + o
    h2 = rms(x1, ln2_w)
    g = h2 @ w_gate
    mlp = (g / (1.0 + np.exp(-g)) * (h2 @ w_up)) @ w_down
    delta = (x1 + mlp - sel) * gate[:, None]
    out = x.copy()
    out[bidx, tidx] += delta
    return out.astype(np.float32)


def kernel(hidden_states, original, posterior, prior, position_ids, w_router,
           ln1_w, ln2_w, wq, wk, wv, wo, w_gate, w_up, w_down, k):
    hidden_states = np.asarray(hidden_states, dtype=np.float32)
    original = np.asarray(original, dtype=np.float32)
    posterior = np.asarray(posterior, dtype=np.float32)
    prior = np.asarray(prior, dtype=np.float32)
    position_ids = np.asarray(position_ids)
    w_router = np.asarray(w_router, dtype=np.float32)
    ln1_w = np.asarray(ln1_w, dtype=np.float32)
    ln2_w = np.asarray(ln2_w, dtype=np.float32)
    wq_, wk_, wv_, wo_ = (np.asarray(a, dtype=np.float32)
                          for a in (wq, wk, wv, wo))
    w_gate_, w_up_, w_down_ = (np.asarray(a, dtype=np.float32)
                               for a in (w_gate, w_up, w_down))
    kk = int(np.asarray(k))

    if (hidden_states.shape != (B, S, D) or kk * B != T):
        return _kernel_numpy_fallback(
            hidden_states, original, posterior, prior, position_ids, w_router,
            ln1_w, ln2_w, wq_, wk_, wv_, wo_, w_gate_, w_up_, w_down_, kk)

    scores, _ = run_scoring(original, posterior, prior, w_router)
    signal = 1.0 / (1.0 + np.exp(-scores.astype(np.float64)))
    idx = np.sort(np.argpartition(-scores, kk, axis=-1)[:, :kk], axis=-1)
    bidx = np.repeat(np.arange(B), kk)
    tidx = idx.reshape(-1)
    sel = np.ascontiguousarray(hidden_states[bidx, tidx])
    gate = signal[bidx, tidx].astype(np.float32)
    pos = position_ids[bidx, tidx]

    delta, _ = run_decoder(sel, pos, gate, ln1_w, wq_, wk_, wv_, wo_,
                           ln2_w, w_gate_, w_up_, w_down_)

    out = hidden_states.copy()
    out[bidx, tidx] += delta
    return out



# revision 8
# speedup vs baseline: 14.7458x; 14.7458x over previous
"""Trainium2 Bass kernel for nn_DTFDynamicLayer (moe_routing).

Self-contained: takes FULL inputs, returns FULL output. Two SPMD NEFFs on 8
NeuronCores:
  NEFF1 (scoring): token-parallel router scores (fp32 — top-k boundaries are
    ~6e-4 apart, bf16 would flip selections).
  Host: top-k per batch row (argpartition), gather selected tokens.
  NEFF2 (decoder): Qwen2 block on the T=B*k=2048 selected tokens.
    Attention is head-parallel (2 Q heads + 1 KV head per core); the o-proj
    full-D partial is ReduceScattered over the TOKEN axis (the only
    collective, 1MB out), after which each core owns 256 tokens and runs the
    ENTIRE MLP for them locally (full gate/up/down weights streamed from
    HBM under the matmuls, P kept in SBUF) — down-proj output is final, no
    second collective. RMS ln weights folded into the weight slices; rms r1
    and rope tables folded host-side; r2 computed on device.
  Host: scatter-add gated delta into a copy of hidden_states.
"""
import sys
sys.path.insert(0, "/opt/trn_rl_repo")
import math
import numpy as np
import ml_dtypes

import jax
from jax.sharding import Mesh, PartitionSpec
from jax.experimental.shard_map import shard_map

import concourse.bacc as bacc
import concourse.mybir as mybir
import concourse.tile as tile
from concourse.bass_utils import run_bass_kernel_spmd
from concourse.bass2jax import (_bass_exec_p, partition_id_tensor,
                                install_neuronx_cc_hook)


def _make_runner(nc, n_cores=8):
    """Persistent jitted shard_map executor for an SPMD Bass program."""
    install_neuronx_cc_hook()
    pname = nc.partition_id_tensor.name if nc.partition_id_tensor else None
    in_names, out_names, out_avals, zero_outs = [], [], [], []
    for alloc in nc.m.functions[0].allocations:
        if not isinstance(alloc, mybir.MemoryLocationSet):
            continue
        name = alloc.memorylocations[0].name
        if alloc.kind == "ExternalInput":
            if name != pname:
                in_names.append(name)
        elif alloc.kind == "ExternalOutput":
            shape = tuple(alloc.tensor_shape)
            dtype = mybir.dt.np(alloc.dtype)
            out_names.append(name)
            out_avals.append(jax.core.ShapedArray(shape, dtype))
            zero_outs.append(np.zeros(shape, dtype))
    all_in = list(in_names) + list(out_names)
    if pname is not None:
        all_in.append(pname)

    def _body(*args):
        operands = list(args)
        if pname is not None:
            operands.append(partition_id_tensor())
        return tuple(_bass_exec_p.bind(
            *operands, out_avals=tuple(out_avals), in_names=tuple(all_in),
            out_names=tuple(out_names), lowering_input_output_aliases=(),
            sim_require_finite=True, sim_require_nnan=True, nc=nc))

    devices = jax.devices()[:n_cores]
    mesh = Mesh(np.asarray(devices), ("core",))
    nin = len(in_names) + len(out_avals)
    sharded = jax.jit(
        shard_map(_body, mesh=mesh,
                  in_specs=(PartitionSpec("core"),) * nin,
                  out_specs=(PartitionSpec("core"),) * len(out_avals),
                  check_rep=False),
        keep_unused=True)
    concat_zeros = [np.zeros((n_cores * z.shape[0], *z.shape[1:]), z.dtype)
                    for z in zero_outs]

    def run(in_maps):
        concat_in = [np.concatenate([np.asarray(in_maps[c][nm])
                                     for c in range(n_cores)], axis=0)
                     for nm in in_names]
        outs = sharded(*concat_in, *concat_zeros)
        return [{nm: np.asarray(outs[i]).reshape(n_cores, *out_avals[i].shape)[c]
                 for i, nm in enumerate(out_names)}
                for c in range(n_cores)]

    return run

N_CORES = 8
B, S = 4, 4096
T = 2048
D = 2048
HD = 128
DFF = 8192
NK = D // 128            # 16 D-chunks
TC = T // 512            # 4 q-chunks
NTOK = T // 128          # 16 token blocks
TG = T // N_CORES        # 256 tokens per core for the MLP
NM = DFF // 128          # 64 dff m-tiles
TOKC = B * S // N_CORES  # scoring tokens per core
NCH = TOKC // 128
EPS = 1e-6
SM_SCALE = 1.0 / math.sqrt(HD)
ROPE_THETA = 10000.0

f32 = mybir.dt.float32
bf16 = mybir.dt.bfloat16
AF = mybir.ActivationFunctionType
OP = mybir.AluOpType
BF = ml_dtypes.bfloat16

_cache = {}
BENCH_MODE = False


def _ext_in(nc, name, shape, dtype, reg):
    """ExternalInput normally; internal DRAM tensor in BENCH_MODE."""
    if BENCH_MODE:
        t = nc.dram_tensor(name, shape, dtype)
        reg.append(t)
        return t
    return nc.dram_tensor(name, shape, dtype, kind="ExternalInput")


def _bench_init(nc, tc, pool, reg):
    """Fill fake-input internal tensors with a small constant (bench mode)."""
    if not reg:
        return
    zt = pool.tile([128, 2048], bf16, tag="benchz", name="benchinit")
    nc.vector.memset(zt[:], 0.01)
    ztf = pool.tile([128, 2048], f32, tag="benchzf", name="benchinitf")
    nc.vector.memset(ztf[:], 0.01)
    n = 0
    for t in reg:
        src_t = ztf if t.dtype == f32 else zt
        rows, cols = (t.shape if len(t.shape) == 2 else (t.shape[0], 1))
        for r in range(0, rows, 128):
            rr = min(128, rows - r)
            for cstart in range(0, cols, 2048):
                cc = min(2048, cols - cstart)
                eng = nc.sync if n % 2 == 0 else nc.scalar
                n += 1
                eng.dma_start(out=t[r:r + rr, cstart:cstart + cc],
                              in_=src_t[0:rr, 0:cc])


# ======================= NEFF1: scoring =======================
def build_scoring(reps=1):
    nc = bacc.Bacc("TRN2", target_bir_lowering=False, debug=False,
                   num_devices=N_CORES)
    breg = []
    orig = _ext_in(nc, "orig", [TOKC, D], f32, breg)
    dsur = _ext_in(nc, "dsur", [TOKC, D], f32, breg)
    wb = _ext_in(nc, "wb", [128, D], f32, breg)
    scores = nc.dram_tensor("scores", [TOKC], f32, kind="ExternalOutput")

    with tile.TileContext(nc) as tc:
        with tc.tile_pool(name="io", bufs=3) as io, \
             tc.tile_pool(name="scratch", bufs=2) as scratch, \
             tc.tile_pool(name="bench0", bufs=1) as bench0, \
             tc.tile_pool(name="acc", bufs=2) as accp:
            _bench_init(nc, tc, bench0, breg)
            for rep in range(reps):
                wb_t = accp.tile([128, D], f32, tag="wb", name=f"wb{rep}")
                nc.sync.dma_start(out=wb_t[:], in_=wb[:, :])
                ss = accp.tile([128, NCH], f32, tag="ss", name=f"ss{rep}")
                dot = accp.tile([128, NCH], f32, tag="dot", name=f"dot{rep}")

                for i in range(NCH):
                    o_t = io.tile([128, D], f32, tag="o", name=f"o{rep}_{i}")
                    d_t = io.tile([128, D], f32, tag="p", name=f"p{rep}_{i}")
                    sl = slice(i * 128, (i + 1) * 128)
                    nc.sync.dma_start(out=o_t[:], in_=orig[sl, :])
                    nc.scalar.dma_start(out=d_t[:], in_=dsur[sl, :])

                    sq_t = scratch.tile([128, D], bf16, tag="sq",
                                        name=f"sq{rep}_{i}")
                    nc.scalar.activation(sq_t[:], d_t[:], AF.Square,
                                         accum_out=ss[:, i:i + 1])
                    pr_t = scratch.tile([128, D], bf16, tag="pr",
                                        name=f"pr{rep}_{i}")
                    nc.vector.scalar_tensor_tensor(pr_t[:], o_t[:], 1.0, wb_t[:],
                                                   op0=OP.mult, op1=OP.mult,
                                                   accum_out=dot[:, i:i + 1])

                sc = accp.tile([128, NCH], f32, tag="sc", name=f"sc{rep}")
                nc.vector.scalar_tensor_tensor(sc[:], ss[:], 0.5 / D, dot[:],
                                               op0=OP.mult, op1=OP.add)
                nc.sync.dma_start(out=scores.rearrange("(n p) -> p n", p=128),
                                  in_=sc[:])
    nc.compile()
    return nc


# ======================= NEFF2: decoder =======================
def build_decoder(reps=1):
    nc = bacc.Bacc("TRN2", target_bir_lowering=False, debug=False,
                   num_devices=N_CORES)
    breg = []
    xT = _ext_in(nc, "xT", [D, T], bf16, breg)
    xg_in = _ext_in(nc, "xg_in", [D, TG], bf16, breg)
    wq_in = _ext_in(nc, "wq_in", [128, NK * 256], bf16, breg)
    wk_in = _ext_in(nc, "wk_in", [128, NK * 128], bf16, breg)
    wv_in = _ext_in(nc, "wv_in", [128, NK * 128], bf16, breg)
    wo_in = _ext_in(nc, "wo_in", [128, 2 * D], bf16, breg)
    wg_in = _ext_in(nc, "wg_in", [128, NM * NK * 128], bf16, breg)
    wu_in = _ext_in(nc, "wu_in", [128, NM * NK * 128], bf16, breg)
    wd_in = _ext_in(nc, "wd_in", [128, NK * NM * 128], bf16, breg)
    cos_in = _ext_in(nc, "cos_in", [128, T], bf16, breg)
    sin_in = _ext_in(nc, "sin_in", [128, T], bf16, breg)
    r1c_in = _ext_in(nc, "r1c_in", [128, NTOK], f32, breg)
    gate_in = _ext_in(nc, "gate_in", [128, TG], bf16, breg)
    delta = nc.dram_tensor("delta", [D, TG], f32, kind="ExternalOutput")
    o_part = nc.dram_tensor("o_part", [N_CORES, D, TG], bf16)
    o_red = nc.dram_tensor("o_red", [D, TG], bf16)
    RG = [list(range(N_CORES))]

    with tile.TileContext(nc) as tc:
        with tc.tile_pool(name="wres", bufs=1) as wres, \
             tc.tile_pool(name="xch", bufs=NK) as xch, \
             tc.tile_pool(name="att", bufs=1) as att, \
             tc.tile_pool(name="ws", bufs=2) as ws, \
             tc.tile_pool(name="rsc", bufs=2) as rsc, \
             tc.tile_pool(name="mlp", bufs=1) as mlp, \
             tc.tile_pool(name="sm", bufs=3) as sm, \
             tc.tile_pool(name="ev", bufs=2) as ev, \
             tc.tile_pool(name="bench0", bufs=1) as bench0, \
             tc.tile_pool(name="ps", bufs=3, space="PSUM") as ps, \
             tc.tile_pool(name="psden", bufs=1, space="PSUM") as psden, \
             tc.tile_pool(name="psg", bufs=2, space="PSUM") as psg:

            _bench_init(nc, tc, bench0, breg)
            for rep in range(reps):
                _decoder_body(nc, tc, rep, xT, xg_in, wq_in, wk_in, wv_in,
                              wo_in, wg_in, wu_in, wd_in, cos_in, sin_in,
                              r1c_in, gate_in, delta, o_part, o_red, RG,
                              wres, xch, att, ws, rsc, mlp, sm, ev,
                              ps, psden, psg)
    nc.compile()
    return nc


def _decoder_body(nc, tc, rep, xT, xg_in, wq_in, wk_in, wv_in, wo_in, wg_in,
                  wu_in, wd_in, cos_in, sin_in, r1c_in, gate_in, delta,
                  o_part, o_red, RG, wres, xch, att, ws, rsc, mlp, sm, ev,
                  ps, psden, psg):
    # ---- resident small tensors ----
    wq_sb = wres.tile([128, NK * 256], bf16, tag="wq", name=f"wq{rep}")
    nc.sync.dma_start(out=wq_sb[:], in_=wq_in[:, :])
    wk_sb = wres.tile([128, NK * 128], bf16, tag="wk", name=f"wk{rep}")
    nc.sync.dma_start(out=wk_sb[:], in_=wk_in[:, :])
    wv_sb = wres.tile([128, NK * 128], bf16, tag="wv", name=f"wv{rep}")
    nc.sync.dma_start(out=wv_sb[:], in_=wv_in[:, :])
    wo_sb = wres.tile([128, 2 * D], bf16, tag="wo", name=f"wo{rep}")
    nc.sync.dma_start(out=wo_sb[:], in_=wo_in[:, :])
    r1c_sb = wres.tile([128, NTOK], f32, tag="r1c", name=f"r1c{rep}")
    nc.sync.dma_start(out=r1c_sb[:], in_=r1c_in[:, :])
    gate_sb = wres.tile([128, TG], bf16, tag="gateb", name=f"gate{rep}")
    nc.sync.dma_start(out=gate_sb[:], in_=gate_in[:, :])
    ones_sb = wres.tile([128, 1], bf16, tag="ones", name=f"ones{rep}")
    nc.vector.memset(ones_sb[:], 1.0)
    cos_sb = ws.tile([128, T], bf16, tag="wsa", name=f"cos{rep}")
    nc.scalar.dma_start(out=cos_sb[:], in_=cos_in[:, :])
    sin_sb = ws.tile([128, T], bf16, tag="wsb", name=f"sin{rep}")
    nc.scalar.dma_start(out=sin_sb[:], in_=sin_in[:, :])

    # ---- x chunks (full T, for QKV + attention) ----
    x_sb = []
    for k in range(NK):
        xk = xch.tile([128, T], bf16, tag="x", name=f"x{rep}_{k}")
        eng = nc.sync if k % 2 == 0 else nc.scalar
        eng.dma_start(out=xk[:], in_=xT[k * 128:(k + 1) * 128, :])
        x_sb.append(xk)

    # ---- Q/K projections ----
    qt = [att.tile([128, T], bf16, tag=f"qo{h}", name=f"qraw{rep}{h}")
          for h in range(2)]
    kt = att.tile([128, T], bf16, tag="kr2", name=f"kraw{rep}")
    for h in range(2):
        for n in range(TC):
            acc = ps.tile([128, 512], f32, tag="acc", name=f"qa{rep}{h}{n}")
            for k in range(NK):
                nc.tensor.matmul(
                    acc[:],
                    wq_sb[:, k * 256 + h * 128: k * 256 + (h + 1) * 128],
                    x_sb[k][:, n * 512:(n + 1) * 512],
                    start=(k == 0), stop=(k == NK - 1))
            if n % 2 == 0:
                nc.scalar.copy(qt[h][:, n * 512:(n + 1) * 512], acc[:])
            else:
                nc.vector.tensor_scalar(qt[h][:, n * 512:(n + 1) * 512],
                                        acc[:], 1.0, None, op0=OP.mult)
    for n in range(TC):
        acc = ps.tile([128, 512], f32, tag="acc", name=f"ka{rep}{n}")
        for k in range(NK):
            nc.tensor.matmul(
                acc[:], wk_sb[:, k * 128:(k + 1) * 128],
                x_sb[k][:, n * 512:(n + 1) * 512],
                start=(k == 0), stop=(k == NK - 1))
        if n % 2 == 0:
            nc.vector.tensor_scalar(kt[:, n * 512:(n + 1) * 512],
                                    acc[:], 1.0, None, op0=OP.mult)
        else:
            nc.scalar.copy(kt[:, n * 512:(n + 1) * 512], acc[:])

    # ---- V projection (r1 fused; token-partition layout) ----
    v_sb = att.tile([128, NTOK * 128], bf16, tag="vg", name=f"vg{rep}")
    for j in range(NTOK):
        acc = ps.tile([128, 512], f32, tag="acc", name=f"va{rep}{j}")
        for k in range(NK):
            nc.tensor.matmul(
                acc[:, 0:128], x_sb[k][:, j * 128:(j + 1) * 128],
                wv_sb[:, k * 128:(k + 1) * 128],
                start=(k == 0), stop=(k == NK - 1))
        nc.vector.tensor_scalar(
            v_sb[:, j * 128:(j + 1) * 128], acc[:, 0:128],
            r1c_sb[:, j:j + 1], None, op0=OP.mult)

    # ---- rope (in place: qt/kt tiles become the roped values) ----
    for raw in (qt[0], qt[1], kt):
        swp = rsc.tile([128, T], bf16, tag="swp", name=f"swp{rep}")
        nc.sync.dma_start(out=swp[0:64, :], in_=raw[64:128, :])
        nc.sync.dma_start(out=swp[64:128, :], in_=raw[0:64, :])
        t1 = rsc.tile([128, T], bf16, tag="t1", name=f"t1{rep}")
        nc.vector.tensor_tensor(t1[:], raw[:], cos_sb[:], op=OP.mult)
        nc.vector.tensor_tensor(swp[:], swp[:], sin_sb[:], op=OP.mult)
        nc.vector.tensor_tensor(raw[:], t1[:], swp[:], op=OP.add)

    # ---- attention + per-chunk o-proj -> o_part (token-grouped) ----
    for c in range(TC):
        ot = [None, None]
        for h in range(2):
            o_acc = ps.tile([128, 512], f32, tag="oacc", bufs=2,
                             name=f"oa{rep}{h}{c}")
            den = psden.tile([1, 512], f32, tag="den", name=f"dn{rep}{h}{c}")
            nj = 4 * c + 4
            for j in range(nj):
                s_ps = ps.tile([128, 512], f32, tag="acc",
                               name=f"s{rep}{h}{c}{j}")
                nc.tensor.matmul(
                    s_ps[:], kt[:, j * 128:(j + 1) * 128],
                    qt[h][:, c * 512:(c + 1) * 512],
                    start=True, stop=True)
                e = sm.tile([128, 512], bf16, tag="exp", name=f"e{rep}")
                nc.scalar.activation(e[:], s_ps[:], AF.Exp, scale=SM_SCALE)
                if j >= 4 * c:
                    v_ = j - 4 * c
                    e2 = sm.tile([128, 512], bf16, tag="exp2", bufs=2,
                                 name=f"e2{rep}")
                    nc.gpsimd.affine_select(
                        e2[:], e[:], pattern=[[1, 512]],
                        compare_op=OP.is_ge, fill=0.0,
                        base=-v_ * 128, channel_multiplier=-1)
                    e = e2
                nc.tensor.matmul(o_acc[:], v_sb[:, j * 128:(j + 1) * 128],
                                 e[:], start=(j == 0), stop=(j == nj - 1))
                nc.tensor.matmul(den[:], ones_sb[:], e[:],
                                 start=(j == 0), stop=(j == nj - 1))
            den_sb = ev.tile([1, 512], f32, tag="densb", name=f"dsb{rep}")
            nc.vector.reciprocal(den_sb[:], den[:])
            den_b = ev.tile([128, 512], f32, tag="denb", name=f"db{rep}")
            nc.gpsimd.partition_broadcast(den_b[:], den_sb[:])
            oth = ev.tile([128, 512], bf16, tag=f"ot{h}", name=f"ot{rep}{h}{c}")
            nc.vector.tensor_tensor(oth[:], o_acc[:], den_b[:], op=OP.mult)
            ot[h] = oth
        for m in range(NK):
            acc = ps.tile([128, 512], f32, tag="acc", name=f"op{rep}{m}{c}")
            nc.tensor.matmul(acc[:], wo_sb[:, m * 128:(m + 1) * 128],
                             ot[0][:], start=True, stop=False)
            nc.tensor.matmul(acc[:], wo_sb[:, D + m * 128: D + (m + 1) * 128],
                             ot[1][:], start=False, stop=True)
            ob = ev.tile([128, 512], bf16, tag="ob", bufs=4, name=f"ob{rep}")
            if m % 2 == 0:
                nc.vector.tensor_scalar(ob[:], acc[:], 1.0, None, op0=OP.mult)
            else:
                nc.scalar.copy(ob[:], acc[:])
            e0 = nc.sync if m % 2 == 0 else nc.scalar
            e1 = nc.scalar if m % 2 == 0 else nc.sync
            e0.dma_start(
                out=o_part[2 * c, m * 128:(m + 1) * 128, :], in_=ob[:, 0:256])
            e1.dma_start(
                out=o_part[2 * c + 1, m * 128:(m + 1) * 128, :],
                in_=ob[:, 256:512])

    # ---- the only collective: token-axis ReduceScatter of o ----
    nc.gpsimd.collective_compute(
        "ReduceScatter", OP.add, replica_groups=RG,
        ins=[o_part[:, :, :]], outs=[o_red[:, :]])

    # ---- x1 = xg + o_red (my 256 tokens); r2 ----
    x1_sb, or_sb = [], []
    for k in range(NK):
        xg = mlp.tile([128, TG], bf16, tag=f"xg{k}", name=f"xg{rep}{k}")
        eng = nc.sync if k % 2 == 0 else nc.scalar
        eng.dma_start(out=xg[:], in_=xg_in[k * 128:(k + 1) * 128, :])
        ork = mlp.tile([128, TG], bf16, tag=f"or{k}", name=f"or{rep}{k}")
        eng2 = nc.scalar if k % 2 == 0 else nc.sync
        eng2.dma_start(out=ork[:], in_=o_red[k * 128:(k + 1) * 128, :])
        nc.vector.tensor_tensor(xg[:], xg[:], ork[:], op=OP.add)
        x1_sb.append(xg)
        or_sb.append(ork)
    r2ps = psden.tile([1, TG], f32, tag="den", name=f"r2ps{rep}")
    for k in range(NK):
        sq = sm.tile([128, TG], bf16, tag="sq", bufs=2, name=f"sq{rep}{k}")
        nc.scalar.activation(sq[:], x1_sb[k][:], AF.Square)
        nc.tensor.matmul(r2ps[:], ones_sb[:], sq[:],
                         start=(k == 0), stop=(k == NK - 1))
    mn = ev.tile([1, TG], f32, tag="r2mn", name=f"r2mn{rep}")
    nc.vector.tensor_scalar(mn[:], r2ps[:], 1.0 / D, EPS,
                            op0=OP.mult, op1=OP.add)
    rc = ev.tile([1, TG], f32, tag="r2rc", name=f"r2rc{rep}")
    nc.vector.reciprocal(rc[:], mn[:])
    r2row = ev.tile([1, TG], bf16, tag="r2row", name=f"r2row{rep}")
    nc.scalar.activation(r2row[:], rc[:], AF.Sqrt)
    r2b = ev.tile([128, TG], bf16, tag="r2b", name=f"r2b{rep}")
    nc.gpsimd.partition_broadcast(r2b[:], r2row[:])

    # ---- gate/up for my tokens: full DFF, weights streamed; P in SBUF ----
    p_sb = [xch.tile([128, T], bf16, tag="x", name=f"p{rep}_{jj}")
            for jj in range(8)]
    for m in range(NM):
        wg_m = ws.tile([128, NK * 128], bf16, tag="wsa", name=f"wg{rep}{m}")
        nc.scalar.dma_start(
            out=wg_m[:], in_=wg_in[:, m * NK * 128:(m + 1) * NK * 128])
        wu_m = ws.tile([128, NK * 128], bf16, tag="wsb", name=f"wu{rep}{m}")
        nc.sync.dma_start(
            out=wu_m[:], in_=wu_in[:, m * NK * 128:(m + 1) * NK * 128])
        gps = psg.tile([128, TG], f32, tag="g", name=f"g{rep}{m}")
        for k in range(NK):
            nc.tensor.matmul(gps[:], wg_m[:, k * 128:(k + 1) * 128],
                             x1_sb[k][:], start=(k == 0), stop=(k == NK - 1))
        ups = psg.tile([128, TG], f32, tag="g", name=f"u{rep}{m}")
        for k in range(NK):
            nc.tensor.matmul(ups[:], wu_m[:, k * 128:(k + 1) * 128],
                             x1_sb[k][:], start=(k == 0), stop=(k == NK - 1))
        gsc = sm.tile([128, TG], bf16, tag="gsc", bufs=2, name=f"gsc{rep}")
        nc.vector.tensor_tensor(gsc[:], gps[:], r2b[:], op=OP.mult)
        gact = sm.tile([128, TG], bf16, tag="gact", bufs=2, name=f"gact{rep}")
        nc.scalar.activation(gact[:], gsc[:], AF.Silu)
        usc = sm.tile([128, TG], bf16, tag="usc", bufs=2, name=f"usc{rep}")
        nc.vector.tensor_tensor(usc[:], ups[:], r2b[:], op=OP.mult)
        nc.vector.tensor_tensor(
            p_sb[m // 8][:, (m % 8) * TG:(m % 8 + 1) * TG],
            gact[:], usc[:], op=OP.mult)

    # ---- down for my tokens: full contraction, output is final ----
    wd_tags = ("qo0", "qo1", "kr2", "vg")
    for m2 in range(NK):
        wd_ch = []
        for q in range(4):
            wdt = att.tile([128, T], bf16, tag=wd_tags[q],
                           name=f"wd{rep}{m2}{q}")
            eng = (nc.sync, nc.scalar, nc.sync, nc.scalar)[q]
            eng.dma_start(
                out=wdt[:],
                in_=wd_in[:, (m2 * 4 + q) * T:(m2 * 4 + q + 1) * T])
            wd_ch.append(wdt)
        acc = psg.tile([128, TG], f32, tag="g", name=f"d{rep}{m2}")
        for k in range(NM):
            nc.tensor.matmul(
                acc[:], wd_ch[k // 16][:, (k % 16) * 128:(k % 16 + 1) * 128],
                p_sb[k // 8][:, (k % 8) * TG:(k % 8 + 1) * TG],
                start=(k == 0), stop=(k == NM - 1))
        dsum = sm.tile([128, TG], f32, tag="dsum", bufs=2, name=f"ds{rep}")
        nc.vector.tensor_tensor(dsum[:], acc[:], or_sb[m2][:], op=OP.add)
        dout = sm.tile([128, TG], f32, tag="dout", bufs=2, name=f"do{rep}")
        nc.vector.tensor_tensor(dout[:], dsum[:], gate_sb[:], op=OP.mult)
        eng = nc.sync if m2 % 2 == 0 else nc.scalar
        eng.dma_start(out=delta[m2 * 128:(m2 + 1) * 128, :], in_=dout[:])


def _get(name, builder):
    if name not in _cache:
        _cache[name] = builder()
    return _cache[name]


def _tile_w(w, kchunks, mblocks=None):
    K, M = w.shape
    if mblocks is None:
        return np.ascontiguousarray(
            w.reshape(kchunks, 128, M).transpose(1, 0, 2)
            .reshape(128, kchunks * M))
    mb = M // mblocks
    return np.ascontiguousarray(
        w.reshape(kchunks, 128, mblocks, mb).transpose(1, 2, 0, 3)
        .reshape(128, mblocks * kchunks * mb))


def _run(nc, in_maps, trace=False):
    key = ("runner", id(nc))
    if key not in _cache:
        _cache[key] = _make_runner(nc)
    results = _cache[key](in_maps)

    class _R:
        pass

    r = _R()
    r.results = results
    r.exec_time_ns = None
    r.profile_json = None
    return r


def run_scoring(original, posterior, prior, w_router, trace=False):
    of = original.reshape(-1, D)
    df = (posterior.reshape(-1, D) - prior.reshape(-1, D))
    wb = np.ascontiguousarray(np.broadcast_to(w_router, (128, D)),
                              dtype=np.float32)
    in_maps = []
    for c in range(N_CORES):
        sl = slice(c * TOKC, (c + 1) * TOKC)
        in_maps.append(dict(orig=of[sl], dsur=df[sl], wb=wb))
    res = _run(_get("scoring", build_scoring), in_maps, trace)
    out = np.concatenate([res.results[c]["scores"] for c in range(N_CORES)])
    return out.reshape(B, S), res


def prep_decoder_in_maps(sel, pos, gate, ln1_w, wq, wk, wv, wo, ln2_w,
                         w_gate, w_up, w_down):
    r1 = 1.0 / np.sqrt((sel.astype(np.float32) ** 2).mean(-1) + EPS)
    xT_bf = np.ascontiguousarray(sel.T.astype(BF))

    inv_freq = 1.0 / (ROPE_THETA ** (np.arange(0, HD, 2, dtype=np.float32) / HD))
    ang = pos[:, None].astype(np.float32) * inv_freq[None, :]
    cos_v = np.cos(ang).T * r1[None, :]
    sin_v = np.sin(ang).T * r1[None, :]
    cos_t = np.ascontiguousarray(np.concatenate([cos_v, cos_v], 0).astype(BF))
    sin_t = np.ascontiguousarray(np.concatenate([-sin_v, sin_v], 0).astype(BF))
    r1c = np.ascontiguousarray(r1.reshape(NTOK, 128).T.astype(np.float32))

    wq_f = (ln1_w[:, None] * wq).astype(np.float32)
    wk_f = (ln1_w[:, None] * wk).astype(np.float32)
    wv_f = (ln1_w[:, None] * wv).astype(np.float32)
    wg_f = (ln2_w[:, None] * w_gate).astype(np.float32)
    wu_f = (ln2_w[:, None] * w_up).astype(np.float32)

    wg_t = _tile_w(wg_f.astype(BF), NK, mblocks=NM)
    wu_t = _tile_w(wu_f.astype(BF), NK, mblocks=NM)
    wd_t = _tile_w(np.asarray(w_down, dtype=np.float32).astype(BF), NM,
                   mblocks=NK)

    in_maps = []
    for c in range(N_CORES):
        kvi = c // 2
        in_maps.append(dict(
            xT=xT_bf,
            xg_in=np.ascontiguousarray(xT_bf[:, c * TG:(c + 1) * TG]),
            wq_in=_tile_w(wq_f[:, c * 256:(c + 1) * 256].astype(BF), NK),
            wk_in=_tile_w(wk_f[:, kvi * 128:(kvi + 1) * 128].astype(BF), NK),
            wv_in=_tile_w(wv_f[:, kvi * 128:(kvi + 1) * 128].astype(BF), NK),
            wo_in=_tile_w(np.asarray(wo, dtype=np.float32)
                          [c * 256:(c + 1) * 256].astype(BF), 2),
            wg_in=wg_t, wu_in=wu_t, wd_in=wd_t,
            cos_in=cos_t, sin_in=sin_t, r1c_in=r1c,
            gate_in=np.ascontiguousarray(np.broadcast_to(
                gate[c * TG:(c + 1) * TG].astype(BF), (128, TG))),
        ))
    return in_maps


def run_decoder(sel, pos, gate, ln1_w, wq, wk, wv, wo, ln2_w, w_gate, w_up,
                w_down, trace=False):
    in_maps = prep_decoder_in_maps(sel, pos, gate, ln1_w, wq, wk, wv, wo,
                                   ln2_w, w_gate, w_up, w_down)
    res = _run(_get("decoder", build_decoder), in_maps, trace)
    delta_T = np.concatenate([res.results[c]["delta"] for c in range(N_CORES)],
                             axis=1)
    return np.ascontiguousarray(delta_T.T, dtype=np.float32), res


def _kernel_numpy_fallback(hidden_states, original, posterior, prior,
                           position_ids, w_router, ln1_w, ln2_w, wq, wk, wv,
                           wo, w_gate, w_up, w_down, k):
    """Pure-numpy reference path (used only if shapes diverge from the spec)."""
    x = hidden_states.astype(np.float64)
    scores = (original.astype(np.float64) @ w_router.astype(np.float64)
              + 0.5 * ((posterior.astype(np.float64)
                        - prior.astype(np.float64)) ** 2).mean(-1))
    signal = 1.0 / (1.0 + np.exp(-scores))
    kk = int(k)
    idx = np.sort(np.argpartition(-scores, kk, axis=-1)[:, :kk], axis=-1)
    bidx = np.repeat(np.arange(x.shape[0]), kk)
    tidx = idx.reshape(-1)
    sel = x[bidx, tidx]
    gate = signal[bidx, tidx]
    pos = position_ids[bidx, tidx]
    Tl = sel.shape[0]
    H, KV = 16, 4

    def rms(v, w):
        return v / np.sqrt((v ** 2).mean(-1, keepdims=True) + EPS) * w

    h = rms(sel, ln1_w)
    q = (h @ wq).reshape(Tl, H, HD)
    k_ = (h @ wk).reshape(Tl, KV, HD)
    v_ = (h @ wv).reshape(Tl, KV, HD)
    inv_freq = 1.0 / (ROPE_THETA ** (np.arange(0, HD, 2) / HD))
    angv = pos[:, None] * inv_freq[None, :]
    cos = np.concatenate([np.cos(angv)] * 2, -1)[:, None, :]
    sin = np.concatenate([np.sin(angv)] * 2, -1)[:, None, :]

    def rope(t):
        t1, t2 = np.split(t, 2, -1)
        return t * cos + np.concatenate([-t2, t1], -1) * sin

    q, k_ = rope(q), rope(k_)
    k_ = np.repeat(k_, H // KV, 1)
    v_ = np.repeat(v_, H // KV, 1)
    att = np.einsum("thd,shd->hts", q, k_) / np.sqrt(HD)
    att = np.where(np.tril(np.ones((Tl, Tl), bool))[None], att, -1e9)
    att = np.exp(att - att.max(-1, keepdims=True))
    att /= att.sum(-1, keepdims=True)
    o = np.einsum("hts,shd->thd", att, v_).reshape(Tl, H * HD) @ wo
    x1 = sel + o
    h2 = rms(x1, ln2_w)
    g = h2 @ w_gate
    mlp = (g / (1.0 + np.exp(-g)) * (h2 @ w_up)) @ w_down
    delta = (x1 + mlp - sel) * gate[:, None]
    out = x.copy()
    out[bidx, tidx] += delta
    return out.astype(np.float32)


def kernel(hidden_states, original, posterior, prior, position_ids, w_router,
           ln1_w, ln2_w, wq, wk, wv, wo, w_gate, w_up, w_down, k):
    hidden_states = np.asarray(hidden_states, dtype=np.float32)
    original = np.asarray(original, dtype=np.float32)
    posterior = np.asarray(posterior, dtype=np.float32)
    prior = np.asarray(prior, dtype=np.float32)
    position_ids = np.asarray(position_ids)
    w_router = np.asarray(w_router, dtype=np.float32)
    ln1_w = np.asarray(ln1_w, dtype=np.float32)
    ln2_w = np.asarray(ln2_w, dtype=np.float32)
    wq_, wk_, wv_, wo_ = (np.asarray(a, dtype=np.float32)
                          for a in (wq, wk, wv, wo))
    w_gate_, w_up_, w_down_ = (np.asarray(a, dtype=np.float32)
                               for a in (w_gate, w_up, w_down))
    kk = int(np.asarray(k))

    if (hidden_states.shape != (B, S, D) or kk * B != T):
        return _kernel_numpy_fallback(
            hidden_states, original, posterior, prior, position_ids, w_router,
            ln1_w, ln2_w, wq_, wk_, wv_, wo_, w_gate_, w_up_, w_down_, kk)

    scores, _ = run_scoring(original, posterior, prior, w_router)
    signal = 1.0 / (1.0 + np.exp(-scores.astype(np.float64)))
    idx = np.sort(np.argpartition(-scores, kk, axis=-1)[:, :kk], axis=-1)
    bidx = np.repeat(np.arange(B), kk)
    tidx = idx.reshape(-1)
    sel = np.ascontiguousarray(hidden_states[bidx, tidx])
    gate = signal[bidx, tidx].astype(np.float32)
    pos = position_ids[bidx, tidx]

    delta, _ = run_decoder(sel, pos, gate, ln1_w, wq_, wk_, wv_, wo_,
                           ln2_w, w_gate_, w_up_, w_down_)

    out = hidden_states.copy()
    out[bidx, tidx] += delta
    return out
